# revision 1
# baseline (speedup 1.0000x reference)
"""MoE top-2 SwiGLU kernel for TRN2, expert-parallel across 8 NeuronCores.

Strategy:
  - Host: fp32 gating (softmax + top-2, exact replication of the reference),
    dispatch = gather each expert's tokens into a padded [d, C] activation
    block (expert parallelism: core e holds expert e's weights only).
  - Device (per core): bf16 SwiGLU MLP over that expert's tokens:
        h = silu(W1 @ x) * (W3 @ x);  out = W2 @ h
    computed entirely transposed ([feature, token] layout) so both matmul
    stages contract on the partition dim with zero on-device transposes.
  - Host: combine = scatter-add weighted expert outputs (fp32).
"""

import numpy as np
import ml_dtypes

import concourse.bass as bass
import concourse.bacc as bacc
import concourse.mybir as mybir
import concourse.tile as tile
from concourse.bass_utils import run_bass_kernel_spmd

BF16 = mybir.dt.bfloat16
F32 = mybir.dt.float32

NUM_EXPERTS = 8
TOP_K = 2
D_MODEL = 1024
D_MLP = 3584
KD = D_MODEL // 128  # 8 contraction chunks over d_model
FC = D_MLP // 128    # 28 chunks over d_mlp

# Populated after each kernel() call so test.py can report device timing.
LAST_RUN = {}

# Overridable for CoreSim checks (Silu not implemented in the interpreter).
ACT_FN = mybir.ActivationFunctionType.Silu

# Tunables (model-swept via TimelineSim; best: X_FIRST + PS2_BUFS=3).
PS1_BUFS = 2
PS2_BUFS = 3
W_BUFS = 4
W2_BUFS = 2
X_FIRST = True  # emit w1/w3 fc=0 DMAs before the xt loads
FC0_KD_OUTER = False  # first f-chunk: kd-outer MM order to overlap xt DMA
REPEAT = 1  # benchmark-only: repeat the whole body N times in one NEFF
PASS_CAP = 1536  # max tokens per core per pass (SBUF residency bound)


def _t_tiles(C):
    tiles = []
    t0 = 0
    while t0 < C:
        tn = min(512, C - t0)
        tiles.append((t0, tn))
        t0 += tn
    return tiles


def _build_bass(C):
    t_tiles = _t_tiles(C)
    nc = bacc.Bacc("TRN2", target_bir_lowering=False, debug=False,
                   num_devices=NUM_EXPERTS)

    xt_d = nc.dram_tensor("xt", [KD, 128, C], BF16, kind="ExternalInput")
    w1_d = nc.dram_tensor("w1t", [FC, 128, D_MODEL], BF16, kind="ExternalInput")
    w3_d = nc.dram_tensor("w3t", [FC, 128, D_MODEL], BF16, kind="ExternalInput")
    w2_d = nc.dram_tensor("w2t", [KD, 128, D_MLP], BF16, kind="ExternalInput")
    out_d = nc.dram_tensor("out", [KD, 128, C], F32, kind="ExternalOutput")

    with tile.TileContext(nc) as tc:
        with (
            tc.tile_pool(name="xpool", bufs=1) as xpool,
            tc.tile_pool(name="wpool", bufs=W_BUFS) as wpool,
            tc.tile_pool(name="w2pool", bufs=W2_BUFS) as w2pool,
            tc.tile_pool(name="hpool", bufs=1) as hpool,
            tc.tile_pool(name="spool", bufs=4) as spool,
            tc.tile_pool(name="opool", bufs=4) as opool,
            tc.tile_pool(name="ps1", bufs=PS1_BUFS, space="PSUM") as ps1,
            tc.tile_pool(name="ps2", bufs=PS2_BUFS, space="PSUM") as ps2,
        ):
            for _rep in range(REPEAT):
                w1_first = w3_first = None
                if X_FIRST:
                    w1_first = wpool.tile([128, D_MODEL], BF16, tag="w1")
                    nc.sync.dma_start(w1_first[:], w1_d[0])
                    w3_first = wpool.tile([128, D_MODEL], BF16, tag="w3")
                    nc.sync.dma_start(w3_first[:], w3_d[0])

                # Resident activations: X^T as 8 chunks of [128 (d), C (tokens)].
                xts = []
                for kd in range(KD):
                    t = xpool.tile([128, C], BF16, tag=f"xt{kd}")
                    nc.sync.dma_start(t[:], xt_d[kd])
                    xts.append(t)

                # Stage 1: h^T[fc] = silu(W1 x)^T * (W3 x)^T, per 128-row f chunk.
                hts = []
                for fc in range(FC):
                    if fc == 0 and X_FIRST:
                        w1, w3 = w1_first, w3_first
                    else:
                        w1 = wpool.tile([128, D_MODEL], BF16, tag="w1")
                        nc.sync.dma_start(w1[:], w1_d[fc])
                        w3 = wpool.tile([128, D_MODEL], BF16, tag="w3")
                        nc.sync.dma_start(w3[:], w3_d[fc])
                    ht = hpool.tile([128, C], BF16, tag=f"h{fc}")
                    head = []
                    if fc == 0 and FC0_KD_OUTER:
                        # kd-outer interleave over the first two token tiles: each
                        # xt chunk is consumed right as its DMA lands instead of
                        # stalling the first psum group on all 8 chunks. Two live
                        # groups per tag fit PS1_BUFS=2.
                        head = t_tiles[:2]
                        ps = [(ps1.tile([128, tn], F32, tag="p1", name=f"p1k{t0}"),
                               ps1.tile([128, tn], F32, tag="p3", name=f"p3k{t0}"))
                              for (t0, tn) in head]
                        for kd in range(KD):
                            for (p1, p3), (t0, tn) in zip(ps, head):
                                nc.tensor.matmul(
                                    p1[:], w1[:, kd * 128:(kd + 1) * 128],
                                    xts[kd][:, t0:t0 + tn],
                                    start=(kd == 0), stop=(kd == KD - 1))
                                nc.tensor.matmul(
                                    p3[:], w3[:, kd * 128:(kd + 1) * 128],
                                    xts[kd][:, t0:t0 + tn],
                                    start=(kd == 0), stop=(kd == KD - 1))
                        for (p1, p3), (t0, tn) in zip(ps, head):
                            s1 = spool.tile([128, tn], F32, tag="s")
                            nc.scalar.activation(s1[:], p1[:], ACT_FN)
                            nc.vector.tensor_mul(ht[:, t0:t0 + tn], s1[:], p3[:])
                    for (t0, tn) in t_tiles[len(head):]:
                        p1 = ps1.tile([128, tn], F32, tag="p1")
                        p3 = ps1.tile([128, tn], F32, tag="p3")
                        for kd in range(KD):
                            nc.tensor.matmul(
                                p1[:], w1[:, kd * 128:(kd + 1) * 128],
                                xts[kd][:, t0:t0 + tn],
                                start=(kd == 0), stop=(kd == KD - 1))
                        for kd in range(KD):
                            nc.tensor.matmul(
                                p3[:], w3[:, kd * 128:(kd + 1) * 128],
                                xts[kd][:, t0:t0 + tn],
                                start=(kd == 0), stop=(kd == KD - 1))
                        s1 = spool.tile([128, tn], F32, tag="s")
                        nc.scalar.activation(s1[:], p1[:], ACT_FN)
                        nc.vector.tensor_mul(ht[:, t0:t0 + tn], s1[:], p3[:])
                    hts.append(ht)

                # Stage 2: out^T[dc] = sum_fc W2T[fc,dc]^T @ h^T[fc]
                for dc in range(KD):
                    w2 = w2pool.tile([128, D_MLP], BF16, tag="w2")
                    nc.sync.dma_start(w2[:], w2_d[dc])
                    for (t0, tn) in t_tiles:
                        po = ps2.tile([128, tn], F32, tag="po")
                        for fc in range(FC):
                            nc.tensor.matmul(
                                po[:], w2[:, fc * 128:(fc + 1) * 128],
                                hts[fc][:, t0:t0 + tn],
                                start=(fc == 0), stop=(fc == FC - 1))
                        ot = opool.tile([128, tn], F32, tag="o")
                        nc.vector.tensor_copy(ot[:], po[:])
                        nc.sync.dma_start(out_d[dc][:, t0:t0 + tn], ot[:])

    nc.compile()
    return nc


def _gate(xt, W_gate):
    """fp32 softmax top-2 gating, matching jax.lax.top_k tie-breaking."""
    logits = xt @ W_gate.T
    m = logits.max(-1, keepdims=True)
    ex = np.exp(logits - m)
    w = ex / ex.sum(-1, keepdims=True)
    top_i = np.argsort(-w, axis=-1, kind="stable")[:, :TOP_K]
    top_w = np.take_along_axis(w, top_i, -1)
    top_w = top_w / top_w.sum(-1, keepdims=True)
    return top_i, top_w.astype(np.float32)


def kernel(x, W_gate, W1, W3, W2):
    x = np.asarray(x, dtype=np.float32)
    W_gate = np.asarray(W_gate, dtype=np.float32)
    W1 = np.asarray(W1, dtype=np.float32)
    W3 = np.asarray(W3, dtype=np.float32)
    W2 = np.asarray(W2, dtype=np.float32)

    B, P, D = x.shape
    T = B * P
    xt = x.reshape(T, D)

    top_i, top_w = _gate(xt, W_gate)

    idxs, wts = [], []
    for e in range(NUM_EXPERTS):
        rows, slots = np.nonzero(top_i == e)
        idxs.append(rows)
        wts.append(top_w[rows, slots])

    max_count = max(len(i) for i in idxs)
    # SBUF fits C up to ~2000 (h residency dominates); split into passes if a
    # pathological routing concentrates tokens on few experts.
    n_pass = max(1, -(-max_count // PASS_CAP))
    cap = -(-max_count // n_pass)
    C = max(512, -(-cap // 16) * 16)

    bf = ml_dtypes.bfloat16
    wt_maps = []
    for e in range(NUM_EXPERTS):
        # lhsT tile layouts, pre-tiled on host so device DMAs are contiguous:
        # w1t[fc, dp, kd*128+fp] = W1[e][fc*128+fp, kd*128+dp]
        w1t = np.ascontiguousarray(
            W1[e].T.reshape(KD, 128, FC, 128).transpose(2, 1, 0, 3)
            .reshape(FC, 128, D_MODEL).astype(bf))
        w3t = np.ascontiguousarray(
            W3[e].T.reshape(KD, 128, FC, 128).transpose(2, 1, 0, 3)
            .reshape(FC, 128, D_MODEL).astype(bf))
        # w2t[dc, fp, fc*128+dp] = W2[e][dc*128+dp, fc*128+fp]
        w2t = np.ascontiguousarray(
            W2[e].T.reshape(FC, 128, KD, 128).transpose(2, 1, 0, 3)
            .reshape(KD, 128, D_MLP).astype(bf))
        wt_maps.append({"w1t": w1t, "w3t": w3t, "w2t": w2t})

    nc = _build_bass(C)
    out = np.zeros((T, D), dtype=np.float32)
    for p in range(n_pass):
        in_maps = []
        for e in range(NUM_EXPERTS):
            sel = idxs[e][p * C:(p + 1) * C]
            XT = np.zeros((D, C), dtype=bf)
            XT[:, :len(sel)] = xt[sel].T.astype(bf)
            in_maps.append({
                "xt": np.ascontiguousarray(XT.reshape(KD, 128, C)),
                **wt_maps[e],
            })
        res = run_bass_kernel_spmd(nc, in_maps, list(range(NUM_EXPERTS)))
        LAST_RUN["results"] = res
        LAST_RUN["C"] = C
        LAST_RUN["nc"] = nc
        LAST_RUN["in_maps"] = in_maps
        for e in range(NUM_EXPERTS):
            sel = idxs[e][p * C:(p + 1) * C]
            if len(sel):
                O = np.asarray(res.results[e]["out"]).reshape(D, C)
                w_sel = wts[e][p * C:(p + 1) * C]
                out[sel] += w_sel[:, None] * O[:, :len(sel)].T
    return out.reshape(B, P, D)



# revision 2
# speedup vs baseline: 1.2059x; 1.2059x over previous
"""MoE top-2 SwiGLU kernel for TRN2, expert-parallel across 8 NeuronCores.

Strategy:
  - Host: fp32 gating (softmax + top-2, exact replication of the reference),
    dispatch = gather each expert's tokens into a padded [d, C] activation
    block (expert parallelism: core e holds expert e's weights only).
  - Device (per core): fp8 SwiGLU MLP over that expert's tokens using
    DoubleRow matmuls (2 fp8 contraction rows per PE cell per cycle, 0.5
    cycles/column for K=256 vs bf16's 1.0 for K=128). Precision is held at
    ~bf16 level with a 3-term split per matmul: each operand is represented
    as hi + lo (both e4m3, lo = RNE residual of hi), and the product
    W·x ~= Whi·xhi + Wlo·xhi + Whi·xlo (the lo·lo term, ~0.07% relative, is
    dropped). 3 terms at 0.25 cyc/col/K128 each = 0.75x the bf16 cycle
    count; measured end-to-end rel err ~2e-3 (vs 4e-3 for bf16).
  - Host: combine = scatter-add weighted expert outputs (fp32).

Scales (e4m3 min normal is 2^-6, so operands are pre-scaled into range):
  W1/W3/W2 stored as fp8(64*W); x at natural scale; h stored as fp8(4*h).
  psum1 = 64*h1 -> silu(psum/64); a = s1*(1/16)*psum3 = 4*h;
  psum_out = (64*W2)*(4*h) = 256*out -> copy with scale 1/256.
"""

import numpy as np
import ml_dtypes

import concourse.bass as bass
import concourse.bacc as bacc
import concourse.mybir as mybir
import concourse.tile as tile
from concourse.bass_utils import run_bass_kernel_spmd

FP8 = mybir.dt.float8e4
F32 = mybir.dt.float32
E4 = ml_dtypes.float8_e4m3  # TRN fp8e4 semantics (max 240); our values << 240

NUM_EXPERTS = 8
TOP_K = 2
D_MODEL = 1024
D_MLP = 3584
KD = D_MODEL // 128  # 8 contraction chunks over d_model
FC = D_MLP // 128    # 28 chunks over d_mlp
DR = mybir.MatmulPerfMode.DoubleRow

# Populated after each kernel() call so test.py can report device timing.
LAST_RUN = {}

# Overridable for CoreSim checks (Silu not implemented in the interpreter).
ACT_FN = mybir.ActivationFunctionType.Silu
COPY_FN = mybir.ActivationFunctionType.Copy

PS1_BUFS = 2
PS2_BUFS = 3
W_BUFS = 4
W2_BUFS = 2
X_FIRST = True  # emit fc=0 weight DMAs before the x loads
TN = 256        # token tile (DoubleRow moving AP = 2*TN <= 512)
PASS_CAP = 1536  # max tokens per core per pass (SBUF residency bound)


def _t_tiles(C):
    tiles = []
    t0 = 0
    while t0 < C:
        tn = min(TN, C - t0)
        tiles.append((t0, tn))
        t0 += tn
    return tiles


def _build_bass(C):
    t_tiles = _t_tiles(C)
    nc = bacc.Bacc("TRN2", target_bir_lowering=False, debug=False,
                   num_devices=NUM_EXPERTS)

    xhi_d = nc.dram_tensor("xhi", [KD, 128, C], FP8, kind="ExternalInput")
    xlo_d = nc.dram_tensor("xlo", [KD, 128, C], FP8, kind="ExternalInput")
    w1hi_d = nc.dram_tensor("w1hi", [FC, 128, KD, 128], FP8, kind="ExternalInput")
    w1lo_d = nc.dram_tensor("w1lo", [FC, 128, KD, 128], FP8, kind="ExternalInput")
    w3hi_d = nc.dram_tensor("w3hi", [FC, 128, KD, 128], FP8, kind="ExternalInput")
    w3lo_d = nc.dram_tensor("w3lo", [FC, 128, KD, 128], FP8, kind="ExternalInput")
    w2hi_d = nc.dram_tensor("w2hi", [KD, 128, FC, 128], FP8, kind="ExternalInput")
    w2lo_d = nc.dram_tensor("w2lo", [KD, 128, FC, 128], FP8, kind="ExternalInput")
    out_d = nc.dram_tensor("out", [KD, 128, C], F32, kind="ExternalOutput")

    with tile.TileContext(nc) as tc:
        with (
            tc.tile_pool(name="xpool", bufs=1) as xpool,
            tc.tile_pool(name="wpool", bufs=W_BUFS) as wpool,
            tc.tile_pool(name="w2pool", bufs=W2_BUFS) as w2pool,
            tc.tile_pool(name="hpool", bufs=1) as hpool,
            tc.tile_pool(name="spool", bufs=4) as spool,
            tc.tile_pool(name="opool", bufs=4) as opool,
            tc.tile_pool(name="ps1", bufs=PS1_BUFS, space="PSUM") as ps1,
            tc.tile_pool(name="ps2", bufs=PS2_BUFS, space="PSUM") as ps2,
        ):
            first_w = None
            if X_FIRST:
                first_w = []
                for d in (w1hi_d, w1lo_d, w3hi_d, w3lo_d):
                    t = wpool.tile([128, KD, 128], FP8, tag="w1")
                    nc.sync.dma_start(t[:], d[0])
                    first_w.append(t)

            # Resident activations: X^T hi/lo as [128 (d), kd, C (tokens)].
            xh = xpool.tile([128, KD, C], FP8, tag="xhi")
            xl = xpool.tile([128, KD, C], FP8, tag="xlo")
            for kd in range(KD):
                nc.sync.dma_start(xh[:, kd, :], xhi_d[kd])
            for kd in range(KD):
                nc.sync.dma_start(xl[:, kd, :], xlo_d[kd])

            # h^T hi/lo, written per f-chunk, consumed by stage 2.
            hh = hpool.tile([128, FC, C], FP8, tag="hhi")
            hl = hpool.tile([128, FC, C], FP8, tag="hlo")

            # Stage 1: h = silu(W1 x) * (W3 x), per 128-row f chunk.
            for fc in range(FC):
                if fc == 0 and X_FIRST:
                    w1h, w1l, w3h, w3l = first_w
                else:
                    w1h = wpool.tile([128, KD, 128], FP8, tag="w1")
                    nc.sync.dma_start(w1h[:], w1hi_d[fc])
                    w1l = wpool.tile([128, KD, 128], FP8, tag="w1")
                    nc.sync.dma_start(w1l[:], w1lo_d[fc])
                    w3h = wpool.tile([128, KD, 128], FP8, tag="w3")
                    nc.sync.dma_start(w3h[:], w3hi_d[fc])
                    w3l = wpool.tile([128, KD, 128], FP8, tag="w3")
                    nc.sync.dma_start(w3l[:], w3lo_d[fc])
                for (t0, tn) in t_tiles:
                    p1 = ps1.tile([128, tn], F32, tag="p1")
                    idx = 0
                    for (wt, xt_) in ((w1h, xh), (w1l, xh), (w1h, xl)):
                        for j in range(KD // 2):
                            nc.tensor.matmul(
                                p1[:], wt[:, 2 * j:2 * j + 2, :],
                                xt_[:, 2 * j:2 * j + 2, t0:t0 + tn],
                                start=(idx == 0), stop=(idx == 11),
                                perf_mode=DR)
                            idx += 1
                    p3 = ps1.tile([128, tn], F32, tag="p3")
                    idx = 0
                    for (wt, xt_) in ((w3h, xh), (w3l, xh), (w3h, xl)):
                        for j in range(KD // 2):
                            nc.tensor.matmul(
                                p3[:], wt[:, 2 * j:2 * j + 2, :],
                                xt_[:, 2 * j:2 * j + 2, t0:t0 + tn],
                                start=(idx == 0), stop=(idx == 11),
                                perf_mode=DR)
                            idx += 1
                    s1 = spool.tile([128, tn], F32, tag="s")
                    nc.scalar.activation(s1[:], p1[:], ACT_FN, scale=1.0 / 64)
                    a = spool.tile([128, tn], F32, tag="a")
                    nc.vector.scalar_tensor_tensor(
                        a[:], s1[:], 1.0 / 16, p3[:],
                        mybir.AluOpType.mult, mybir.AluOpType.mult)
                    nc.scalar.activation(hh[:, fc, t0:t0 + tn], a[:], COPY_FN)
                    nc.vector.scalar_tensor_tensor(
                        hl[:, fc, t0:t0 + tn], a[:], 1.0, hh[:, fc, t0:t0 + tn],
                        mybir.AluOpType.mult, mybir.AluOpType.subtract)

            # Stage 2: out^T[dc] = sum_fc W2T[fc,dc]^T @ h^T[fc]
            for dc in range(KD):
                w2h = w2pool.tile([128, FC, 128], FP8, tag="w2h")
                nc.sync.dma_start(w2h[:], w2hi_d[dc])
                w2l = w2pool.tile([128, FC, 128], FP8, tag="w2l")
                nc.sync.dma_start(w2l[:], w2lo_d[dc])
                for (t0, tn) in t_tiles:
                    po = ps2.tile([128, tn], F32, tag="po")
                    idx = 0
                    for (wt, ht_) in ((w2h, hh), (w2l, hh), (w2h, hl)):
                        for j in range(FC // 2):
                            nc.tensor.matmul(
                                po[:], wt[:, 2 * j:2 * j + 2, :],
                                ht_[:, 2 * j:2 * j + 2, t0:t0 + tn],
                                start=(idx == 0), stop=(idx == 41),
                                perf_mode=DR)
                            idx += 1
                    ot = opool.tile([128, tn], F32, tag="o")
                    nc.scalar.activation(ot[:], po[:], COPY_FN, scale=1.0 / 256)
                    nc.sync.dma_start(out_d[dc][:, t0:t0 + tn], ot[:])

    nc.compile()
    return nc


def _gate(xt, W_gate):
    """fp32 softmax top-2 gating, matching jax.lax.top_k tie-breaking."""
    logits = xt @ W_gate.T
    m = logits.max(-1, keepdims=True)
    ex = np.exp(logits - m)
    w = ex / ex.sum(-1, keepdims=True)
    top_i = np.argsort(-w, axis=-1, kind="stable")[:, :TOP_K]
    top_w = np.take_along_axis(w, top_i, -1)
    top_w = top_w / top_w.sum(-1, keepdims=True)
    return top_i, top_w.astype(np.float32)


def _split8(v):
    """hi/lo e4m3 pair: hi = fp8(v), lo = fp8(v - hi)."""
    hi = np.asarray(v, dtype=E4)
    lo = np.asarray(v - hi.astype(np.float32), dtype=E4)
    return hi, lo


def kernel(x, W_gate, W1, W3, W2):
    x = np.asarray(x, dtype=np.float32)
    W_gate = np.asarray(W_gate, dtype=np.float32)
    W1 = np.asarray(W1, dtype=np.float32)
    W3 = np.asarray(W3, dtype=np.float32)
    W2 = np.asarray(W2, dtype=np.float32)

    B, P, D = x.shape
    T = B * P
    xt = x.reshape(T, D)

    top_i, top_w = _gate(xt, W_gate)

    idxs, wts = [], []
    for e in range(NUM_EXPERTS):
        rows, slots = np.nonzero(top_i == e)
        idxs.append(rows)
        wts.append(top_w[rows, slots])

    max_count = max(len(i) for i in idxs)
    # SBUF fits C up to ~1536 (h residency dominates); split into passes if a
    # pathological routing concentrates tokens on few experts.
    n_pass = max(1, -(-max_count // PASS_CAP))
    cap = -(-max_count // n_pass)
    C = max(512, -(-cap // 16) * 16)

    wt_maps = []
    for e in range(NUM_EXPERTS):
        # lhsT tile layouts, pre-tiled on host so device DMAs are contiguous:
        # w1t[fc, dp, kd, m] = 64*W1[e][fc*128+m, kd*128+dp]
        w1t = np.ascontiguousarray(
            W1[e].T.reshape(KD, 128, FC, 128).transpose(2, 1, 0, 3)) * 64.0
        w3t = np.ascontiguousarray(
            W3[e].T.reshape(KD, 128, FC, 128).transpose(2, 1, 0, 3)) * 64.0
        # w2t[dc, fp, fc, m] = 64*W2[e][dc*128+m, fc*128+fp]
        w2t = np.ascontiguousarray(
            W2[e].T.reshape(FC, 128, KD, 128).transpose(2, 1, 0, 3)) * 64.0
        w1hi, w1lo = _split8(w1t)
        w3hi, w3lo = _split8(w3t)
        w2hi, w2lo = _split8(w2t)
        wt_maps.append({"w1hi": w1hi, "w1lo": w1lo, "w3hi": w3hi,
                        "w3lo": w3lo, "w2hi": w2hi, "w2lo": w2lo})

    nc = _build_bass(C)
    out = np.zeros((T, D), dtype=np.float32)
    for p in range(n_pass):
        in_maps = []
        for e in range(NUM_EXPERTS):
            sel = idxs[e][p * C:(p + 1) * C]
            XT = np.zeros((D, C), dtype=np.float32)
            XT[:, :len(sel)] = xt[sel].T
            xhi, xlo = _split8(XT)
            in_maps.append({
                "xhi": np.ascontiguousarray(xhi.reshape(KD, 128, C)),
                "xlo": np.ascontiguousarray(xlo.reshape(KD, 128, C)),
                **wt_maps[e],
            })
        res = run_bass_kernel_spmd(nc, in_maps, list(range(NUM_EXPERTS)))
        LAST_RUN["results"] = res
        LAST_RUN["C"] = C
        LAST_RUN["nc"] = nc
        LAST_RUN["in_maps"] = in_maps
        for e in range(NUM_EXPERTS):
            sel = idxs[e][p * C:(p + 1) * C]
            if len(sel):
                O = np.asarray(res.results[e]["out"]).reshape(D, C)
                w_sel = wts[e][p * C:(p + 1) * C]
                out[sel] += w_sel[:, None] * O[:, :len(sel)].T
    return out.reshape(B, P, D)


# revision 4
# speedup vs baseline: 1.3158x; 1.0911x over previous
"""MoE top-2 SwiGLU kernel for TRN2, expert-parallel across 8 NeuronCores.

Strategy:
  - Host: fp32 gating (softmax + top-2, exact replication of the reference),
    dispatch = gather each expert's tokens into a padded [d, C] activation
    block (expert parallelism: core e holds expert e's weights only).
  - Device (per core): fp8 SwiGLU MLP over that expert's tokens using
    DoubleRow matmuls (2 fp8 contraction rows per PE cell per cycle, 0.5
    cycles/column for K=256 vs bf16's 1.0 for K=128). Precision is held at
    ~bf16 level with a 3-term split per matmul: each operand is represented
    as hi + lo (both e4m3, lo = RNE residual of hi), and the product
    W·x ~= Whi·xhi + Wlo·xhi + Whi·xlo (the lo·lo term, ~0.07% relative, is
    dropped). 3 terms at 0.25 cyc/col/K128 each = 0.75x the bf16 cycle
    count; measured end-to-end rel err ~2e-3 (vs 4e-3 for bf16).
  - Host: combine = scatter-add weighted expert outputs (fp32).

Scales (e4m3 min normal is 2^-6, so operands are pre-scaled into range):
  W1/W3/W2 stored as fp8(64*W); x at natural scale; h stored as fp8(4*h).
  psum1 = 64*h1 -> silu(psum/64); a = s1*(1/16)*psum3 = 4*h;
  psum_out = (64*W2)*(4*h) = 256*out -> copy with scale 1/256.

DMA layout: x ships as one [128, KD, C] tensor per hi/lo (single DMA);
stage-1 weights for one f-chunk ship fused as [128, 4, KD, 128]
(w1hi|w1lo|w3hi|w3lo, single DMA); stage-2 as [128, 2, FC, 128].
"""

import numpy as np
import ml_dtypes

import concourse.bass as bass
import concourse.bacc as bacc
import concourse.mybir as mybir
import concourse.tile as tile
from concourse.bass_utils import run_bass_kernel_spmd

FP8 = mybir.dt.float8e4
F32 = mybir.dt.float32
E4 = ml_dtypes.float8_e4m3  # TRN fp8e4 semantics (max 240); our values << 240

NUM_EXPERTS = 8
TOP_K = 2
D_MODEL = 1024
D_MLP = 3584
KD = D_MODEL // 128  # 8 contraction chunks over d_model
FC = D_MLP // 128    # 28 chunks over d_mlp
DR = mybir.MatmulPerfMode.DoubleRow

# Populated after each kernel() call so test.py can report device timing.
LAST_RUN = {}

# Overridable for CoreSim checks (Silu not implemented in the interpreter).
ACT_FN = mybir.ActivationFunctionType.Silu
COPY_FN = mybir.ActivationFunctionType.Copy

PS1_BUFS = 3
PS2_BUFS = 2
W_BUFS = 4
W2_BUFS = 2
TN = 256        # max token tile (DoubleRow moving AP = 2*TN <= 512)
PASS_CAP = 1536  # max tokens per core per pass (SBUF residency bound)


def _t_tiles(C):
    """Balanced token tiles of width <= TN (avoids a runt tail tile whose
    consumer chain outweighs its PE time and stalls the psum ring)."""
    n = -(-C // TN)
    base, rem = divmod(C, n)
    tiles = []
    t0 = 0
    for i in range(n):
        tn = base + (1 if i < rem else 0)
        tiles.append((t0, tn))
        t0 += tn
    return tiles


def _build_bass(C):
    t_tiles = _t_tiles(C)
    nc = bacc.Bacc("TRN2", target_bir_lowering=False, debug=False,
                   num_devices=NUM_EXPERTS)

    xhi_d = nc.dram_tensor("xhi", [128, KD, C], FP8, kind="ExternalInput")
    xlo_d = nc.dram_tensor("xlo", [128, KD, C], FP8, kind="ExternalInput")
    # fused stage-1 weights: [fc][dp][w1hi|w1lo|w3hi|w3lo][kd][m]
    wa_d = nc.dram_tensor("wa", [FC, 128, 4, KD, 128], FP8, kind="ExternalInput")
    # fused stage-2 weights: [dc][fp][w2hi|w2lo][fc][m]
    w2_d = nc.dram_tensor("w2", [KD, 128, 2, FC, 128], FP8, kind="ExternalInput")
    out_d = nc.dram_tensor("out", [KD, 128, C], F32, kind="ExternalOutput")

    with tile.TileContext(nc) as tc:
        with (
            tc.tile_pool(name="xpool", bufs=1) as xpool,
            tc.tile_pool(name="wpool", bufs=W_BUFS) as wpool,
            tc.tile_pool(name="w2pool", bufs=W2_BUFS) as w2pool,
            tc.tile_pool(name="hpool", bufs=1) as hpool,
            tc.tile_pool(name="spool", bufs=4) as spool,
            tc.tile_pool(name="opool", bufs=4) as opool,
            tc.tile_pool(name="ps1", bufs=PS1_BUFS, space="PSUM") as ps1,
            tc.tile_pool(name="ps2", bufs=PS2_BUFS, space="PSUM") as ps2,
        ):
            # First f-chunk's weights before x so the PE can start the moment
            # x lands; x ships as two whole-tensor DMAs.
            wa0 = wpool.tile([128, 4, KD, 128], FP8, tag="wa")
            nc.sync.dma_start(wa0[:], wa_d[0])
            xh = xpool.tile([128, KD, C], FP8, tag="xhi")
            nc.sync.dma_start(xh[:], xhi_d[:])
            xl = xpool.tile([128, KD, C], FP8, tag="xlo")
            nc.sync.dma_start(xl[:], xlo_d[:])

            # h^T hi/lo, written per f-chunk, consumed by stage 2.
            hh = hpool.tile([128, FC, C], FP8, tag="hhi")
            hl = hpool.tile([128, FC, C], FP8, tag="hlo")

            # Stage 1: h = silu(W1 x) * (W3 x), per 128-row f chunk.
            for fc in range(FC):
                if fc == 0:
                    wa = wa0
                else:
                    wa = wpool.tile([128, 4, KD, 128], FP8, tag="wa")
                    nc.sync.dma_start(wa[:], wa_d[fc])
                for (t0, tn) in t_tiles:
                    p1 = ps1.tile([128, tn], F32, tag="p1")
                    idx = 0
                    for (w, xt_) in ((0, xh), (1, xh), (0, xl)):
                        for j in range(KD // 2):
                            nc.tensor.matmul(
                                p1[:], wa[:, w, 2 * j:2 * j + 2, :],
                                xt_[:, 2 * j:2 * j + 2, t0:t0 + tn],
                                start=(idx == 0), stop=(idx == 11),
                                perf_mode=DR)
                            idx += 1
                    p3 = ps1.tile([128, tn], F32, tag="p3")
                    idx = 0
                    for (w, xt_) in ((2, xh), (3, xh), (2, xl)):
                        for j in range(KD // 2):
                            nc.tensor.matmul(
                                p3[:], wa[:, w, 2 * j:2 * j + 2, :],
                                xt_[:, 2 * j:2 * j + 2, t0:t0 + tn],
                                start=(idx == 0), stop=(idx == 11),
                                perf_mode=DR)
                            idx += 1
                    s1 = spool.tile([128, tn], F32, tag="s")
                    nc.scalar.activation(s1[:], p1[:], ACT_FN, scale=1.0 / 64)
                    a = spool.tile([128, tn], F32, tag="a")
                    nc.vector.scalar_tensor_tensor(
                        a[:], s1[:], 1.0 / 16, p3[:],
                        mybir.AluOpType.mult, mybir.AluOpType.mult)
                    nc.scalar.activation(hh[:, fc, t0:t0 + tn], a[:], COPY_FN)
                    nc.vector.scalar_tensor_tensor(
                        hl[:, fc, t0:t0 + tn], a[:], 1.0, hh[:, fc, t0:t0 + tn],
                        mybir.AluOpType.mult, mybir.AluOpType.subtract)

            # Stage 2: out^T[dc] = sum_fc W2T[fc,dc]^T @ h^T[fc]
            for dc in range(KD):
                w2 = w2pool.tile([128, 2, FC, 128], FP8, tag="w2")
                nc.sync.dma_start(w2[:], w2_d[dc])
                for (t0, tn) in t_tiles:
                    po = ps2.tile([128, tn], F32, tag="po")
                    idx = 0
                    for (w, ht_) in ((0, hh), (1, hh), (0, hl)):
                        for j in range(FC // 2):
                            nc.tensor.matmul(
                                po[:], w2[:, w, 2 * j:2 * j + 2, :],
                                ht_[:, 2 * j:2 * j + 2, t0:t0 + tn],
                                start=(idx == 0), stop=(idx == 41),
                                perf_mode=DR)
                            idx += 1
                    ot = opool.tile([128, tn], F32, tag="o")
                    nc.scalar.activation(ot[:], po[:], COPY_FN, scale=1.0 / 256)
                    nc.sync.dma_start(out_d[dc][:, t0:t0 + tn], ot[:])

    nc.compile()
    return nc


def _gate(xt, W_gate):
    """fp32 softmax top-2 gating, matching jax.lax.top_k tie-breaking."""
    logits = xt @ W_gate.T
    m = logits.max(-1, keepdims=True)
    ex = np.exp(logits - m)
    w = ex / ex.sum(-1, keepdims=True)
    top_i = np.argsort(-w, axis=-1, kind="stable")[:, :TOP_K]
    top_w = np.take_along_axis(w, top_i, -1)
    top_w = top_w / top_w.sum(-1, keepdims=True)
    return top_i, top_w.astype(np.float32)


def _split8(v):
    """hi/lo e4m3 pair: hi = fp8(v), lo = fp8(v - hi)."""
    hi = np.asarray(v, dtype=E4)
    lo = np.asarray(v - hi.astype(np.float32), dtype=E4)
    return hi, lo


def kernel(x, W_gate, W1, W3, W2):
    x = np.asarray(x, dtype=np.float32)
    W_gate = np.asarray(W_gate, dtype=np.float32)
    W1 = np.asarray(W1, dtype=np.float32)
    W3 = np.asarray(W3, dtype=np.float32)
    W2 = np.asarray(W2, dtype=np.float32)

    B, P, D = x.shape
    T = B * P
    xt = x.reshape(T, D)

    top_i, top_w = _gate(xt, W_gate)

    idxs, wts = [], []
    for e in range(NUM_EXPERTS):
        rows, slots = np.nonzero(top_i == e)
        idxs.append(rows)
        wts.append(top_w[rows, slots])

    max_count = max(len(i) for i in idxs)
    # SBUF fits C up to ~1536 (h residency dominates); split into passes if a
    # pathological routing concentrates tokens on few experts.
    n_pass = max(1, -(-max_count // PASS_CAP))
    cap = -(-max_count // n_pass)
    C = max(512, -(-cap // 16) * 16)

    wt_maps = []
    for e in range(NUM_EXPERTS):
        # lhsT tile layouts, pre-tiled on host so device DMAs are contiguous:
        # w1t[fc, dp, kd, m] = 64*W1[e][fc*128+m, kd*128+dp]
        w1t = np.ascontiguousarray(
            W1[e].T.reshape(KD, 128, FC, 128).transpose(2, 1, 0, 3)) * 64.0
        w3t = np.ascontiguousarray(
            W3[e].T.reshape(KD, 128, FC, 128).transpose(2, 1, 0, 3)) * 64.0
        # w2t[dc, fp, fc, m] = 64*W2[e][dc*128+m, fc*128+fp]
        w2t = np.ascontiguousarray(
            W2[e].T.reshape(FC, 128, KD, 128).transpose(2, 1, 0, 3)) * 64.0
        w1hi, w1lo = _split8(w1t)
        w3hi, w3lo = _split8(w3t)
        w2hi, w2lo = _split8(w2t)
        wa = np.ascontiguousarray(
            np.stack([w1hi, w1lo, w3hi, w3lo], axis=2))  # [FC,128,4,KD,128]
        w2f = np.ascontiguousarray(
            np.stack([w2hi, w2lo], axis=2))               # [KD,128,2,FC,128]
        wt_maps.append({"wa": wa, "w2": w2f})

    nc = _build_bass(C)
    out = np.zeros((T, D), dtype=np.float32)
    for p in range(n_pass):
        in_maps = []
        for e in range(NUM_EXPERTS):
            sel = idxs[e][p * C:(p + 1) * C]
            XT = np.zeros((D, C), dtype=np.float32)
            XT[:, :len(sel)] = xt[sel].T
            xhi, xlo = _split8(XT)
            # device x layout: [dp (partition), kd, c]
            in_maps.append({
                "xhi": np.ascontiguousarray(
                    xhi.reshape(KD, 128, C).swapaxes(0, 1)),
                "xlo": np.ascontiguousarray(
                    xlo.reshape(KD, 128, C).swapaxes(0, 1)),
                **wt_maps[e],
            })
        res = run_bass_kernel_spmd(nc, in_maps, list(range(NUM_EXPERTS)))
        LAST_RUN["results"] = res
        LAST_RUN["C"] = C
        LAST_RUN["nc"] = nc
        LAST_RUN["in_maps"] = in_maps
        for e in range(NUM_EXPERTS):
            sel = idxs[e][p * C:(p + 1) * C]
            if len(sel):
                O = np.asarray(res.results[e]["out"]).reshape(D, C)
                w_sel = wts[e][p * C:(p + 1) * C]
                out[sel] += w_sel[:, None] * O[:, :len(sel)].T
    return out.reshape(B, P, D)


# revision 14
# speedup vs baseline: 1.3438x; 1.0213x over previous
"""MoE top-2 SwiGLU kernel for TRN2, expert-parallel across 8 NeuronCores.

Strategy:
  - Host: fp32 gating (softmax + top-2, exact replication of the reference),
    dispatch = gather expert tokens into padded [d, C] activation blocks.
  - Load balance: each core runs TWO fixed-size column blocks (C_A + C_B = C)
    with independent weight inputs, so expert token counts need not fit one
    core. With counts sorted desc, the top k experts take two A-blocks each,
    the bottom k two B-blocks, the middle 8-2k one of each; k and the block
    sizes are chosen per routing to minimize C (>= ceil(T*TOP_K/8), vs
    C = max_count for plain expert parallelism).
  - Device (per core, per block): fp8 SwiGLU MLP using DoubleRow matmuls
    (2 fp8 contraction rows per PE cell per cycle, 0.5 cycles/column for
    K=256 vs bf16's 1.0 for K=128). Precision is held at ~bf16 level with a
    3-term split: operands are hi + lo (both e4m3, lo = RNE residual), and
    W·x ~= Whi·xhi + Wlo·xhi + Whi·xlo (lo·lo, ~0.07% relative, dropped).
    3 terms at 0.25 cyc/col/K128 = 0.75x the bf16 cycle count; measured
    end-to-end rel err ~2e-3 (vs 4e-3 for bf16).
  - Host: combine = scatter-add weighted expert outputs (fp32).

Scales (e4m3 min normal is 2^-6, so operands are pre-scaled into range):
  W1/W3/W2 stored as fp8(64*W); x at natural scale; h stored as fp8(4*h).
  psum1 = 64*h1 -> silu(psum/64); a = s1*(1/16)*psum3 = 4*h;
  psum_out = (64*W2)*(4*h) = 256*out -> copy with scale 1/256.
"""

import numpy as np
import ml_dtypes

import concourse.bass as bass
import concourse.bacc as bacc
import concourse.mybir as mybir
import concourse.tile as tile
from concourse.bass_utils import run_bass_kernel_spmd

FP8 = mybir.dt.float8e4
F32 = mybir.dt.float32
E4 = ml_dtypes.float8_e4m3  # TRN fp8e4 semantics (max 240); our values << 240

NUM_EXPERTS = 8
TOP_K = 2
D_MODEL = 1024
D_MLP = 3584
KD = D_MODEL // 128  # 8 contraction chunks over d_model
FC = D_MLP // 128    # 28 chunks over d_mlp
DR = mybir.MatmulPerfMode.DoubleRow

# Populated after each kernel() call so test.py can report device timing.
LAST_RUN = {}

# Overridable for CoreSim checks (Silu not implemented in the interpreter).
ACT_FN = mybir.ActivationFunctionType.Silu
COPY_FN = mybir.ActivationFunctionType.Copy

PS1_BUFS = 3
PS2_BUFS = 2
W_BUFS = 4
W2_BUFS = 2
TN = 256        # max token tile (DoubleRow moving AP = 2*TN <= 512)
DEFER0 = 3      # first-block fc0 tiles whose xlo-term is deferred (startup)
PASS_CAP = 1536  # max tokens per core per pass (SBUF residency bound)


def _t_tiles(t0, n):
    """Balanced token tiles of width <= TN over [t0, t0+n) (avoids a runt
    tail tile whose consumer chain outweighs its PE time)."""
    if n == 0:
        return []
    m = -(-n // TN)
    base, rem = divmod(n, m)
    tiles = []
    for i in range(m):
        tn = base + (1 if i < rem else 0)
        tiles.append((t0, tn))
        t0 += tn
    return tiles


def _plan_blocks(counts):
    """Choose (C, C_A, C_B, A_slots, B_slots): 8 A-blocks of C_A columns and
    8 B-blocks of C_B columns, each expert covered by exactly 2 blocks."""
    order = sorted(range(NUM_EXPERTS), key=lambda e: -counts[e])
    best = None
    for k in range(5):
        aa, bb = order[:k], order[NUM_EXPERTS - k:] if k else []
        ab = order[k:NUM_EXPERTS - k] if k else order
        ca = max((-(-counts[e] // 2) for e in aa), default=0)
        cb = max((-(-counts[e] // 2) for e in bb), default=0)
        need = max(ca + cb, max((counts[e] for e in ab), default=0))
        C = -(-max(need, 512) // 16) * 16
        if best is None or C < best[0]:
            best = (C, k, ca)
    C, k, ca = best
    aa, bb = order[:k], order[NUM_EXPERTS - k:] if k else []
    ab = order[k:NUM_EXPERTS - k] if k else order
    C_A = max(ca, C // 2)      # give A the larger share (and any slack)
    C_B = C - C_A
    A_slots, B_slots = [], []
    for e in aa:
        n1 = min(C_A, counts[e])
        A_slots += [(e, 0, n1), (e, n1, counts[e] - n1)]
    for e in bb:
        n1 = min(C_B, counts[e])
        B_slots += [(e, 0, n1), (e, n1, counts[e] - n1)]
    for e in ab:
        n1 = min(C_A, counts[e])
        A_slots.append((e, 0, n1))
        B_slots.append((e, n1, counts[e] - n1))
    assert len(A_slots) == NUM_EXPERTS and len(B_slots) == NUM_EXPERTS
    assert all(n <= C_B for (_, _, n) in B_slots)
    return C, C_A, C_B, A_slots, B_slots


def _build_bass(C, C_A):
    tiles_a = _t_tiles(0, C_A)
    tiles_b = _t_tiles(C_A, C - C_A)
    nc = bacc.Bacc("TRN2", target_bir_lowering=False, debug=False,
                   num_devices=NUM_EXPERTS)

    # x split in half along kd so the PE can start on the first half
    xhi_d = nc.dram_tensor("xhi", [2, 128, KD // 2, C], FP8, kind="ExternalInput")
    xlo_d = nc.dram_tensor("xlo", [2, 128, KD // 2, C], FP8, kind="ExternalInput")
    # fused stage-1 weights per block: [fc][dp][w1hi|w1lo|w3hi|w3lo][kd][m]
    wa_ds = [nc.dram_tensor(f"wa_{s}", [FC, 128, 4, KD, 128], FP8,
                            kind="ExternalInput") for s in "ab"]
    # fused stage-2 weights per block: [dc][fp][w2hi|w2lo][fc][m]
    w2_ds = [nc.dram_tensor(f"w2_{s}", [KD, 128, 2, FC, 128], FP8,
                            kind="ExternalInput") for s in "ab"]
    out_d = nc.dram_tensor("out", [KD, 128, C], F32, kind="ExternalOutput")

    blocks = [(wa_ds[0], w2_ds[0], tiles_a), (wa_ds[1], w2_ds[1], tiles_b)]
    blocks = [b for b in blocks if b[2]]

    with tile.TileContext(nc) as tc:
        with (
            tc.tile_pool(name="xpool", bufs=1) as xpool,
            tc.tile_pool(name="wpool", bufs=W_BUFS) as wpool,
            tc.tile_pool(name="w2pool", bufs=W2_BUFS) as w2pool,
            tc.tile_pool(name="hpool", bufs=1) as hpool,
            tc.tile_pool(name="spool", bufs=4) as spool,
            tc.tile_pool(name="opool", bufs=4) as opool,
            tc.tile_pool(name="ps1", bufs=PS1_BUFS, space="PSUM") as ps1,
            tc.tile_pool(name="ps2", bufs=PS2_BUFS, space="PSUM") as ps2,
        ):
            # First f-chunk's weights before x so the PE can start the moment
            # x lands (w1 half first — it is consumed first); x ships as two
            # half-tensor DMAs per hi/lo so matmuls start while x streams.
            wa0 = wpool.tile([128, 4, KD, 128], FP8, tag="wa")
            nc.sync.dma_start(wa0[:, 0:2, :, :], blocks[0][0][0][:, 0:2])
            xh = xpool.tile([128, KD, C], FP8, tag="xhi")
            xl = xpool.tile([128, KD, C], FP8, tag="xlo")
            H = KD // 2
            nc.sync.dma_start(xh[:, :H, :], xhi_d[0])
            nc.sync.dma_start(xh[:, H:, :], xhi_d[1])
            nc.sync.dma_start(wa0[:, 2:4, :, :], blocks[0][0][0][:, 2:4])
            nc.sync.dma_start(xl[:, :H, :], xlo_d[0])
            nc.sync.dma_start(xl[:, H:, :], xlo_d[1])

            # h^T hi/lo, written per f-chunk, consumed by stage 2.
            hh = hpool.tile([128, FC, C], FP8, tag="hhi")
            hl = hpool.tile([128, FC, C], FP8, tag="hlo")

            # Stage 1: h = silu(W1 x) * (W3 x), per 128-row f chunk.
            def mm_group(psum, wa, wlist, t0, tn, start, stop):
                idx = 0
                n = len(wlist) * (KD // 2)
                for (w, xt_) in wlist:
                    for j in range(KD // 2):
                        nc.tensor.matmul(
                            psum[:], wa[:, w, 2 * j:2 * j + 2, :],
                            xt_[:, 2 * j:2 * j + 2, t0:t0 + tn],
                            start=(start and idx == 0),
                            stop=(stop and idx == n - 1), perf_mode=DR)
                        idx += 1

            def consume(p1, p3, fc, t0, tn):
                s1 = spool.tile([128, tn], F32, tag="s")
                nc.scalar.activation(s1[:], p1[:], ACT_FN, scale=1.0 / 64)
                a = spool.tile([128, tn], F32, tag="a")
                nc.vector.scalar_tensor_tensor(
                    a[:], s1[:], 1.0 / 16, p3[:],
                    mybir.AluOpType.mult, mybir.AluOpType.mult)
                nc.scalar.activation(hh[:, fc, t0:t0 + tn], a[:], COPY_FN)
                nc.vector.scalar_tensor_tensor(
                    hl[:, fc, t0:t0 + tn], a[:], 1.0, hh[:, fc, t0:t0 + tn],
                    mybir.AluOpType.mult, mybir.AluOpType.subtract)

            AB1, C1 = [(0, xh), (1, xh)], [(0, xl)]
            AB3, C3 = [(2, xh), (3, xh)], [(2, xl)]
            for bi, (wa_d, _, t_tiles) in enumerate(blocks):
                for fc in range(FC):
                    if bi == 0 and fc == 0:
                        wa = wa0
                        # defer the xlo-dependent term of the first DEFER0
                        # tiles so the PE has xhi-only work while xlo streams.
                        defer = t_tiles[:min(DEFER0, len(t_tiles))]
                        ps = []
                        for (t0, tn) in defer:
                            p1 = ps1.tile([128, tn], F32, tag="p1")
                            mm_group(p1, wa, AB1, t0, tn, True, False)
                            p3 = ps1.tile([128, tn], F32, tag="p3")
                            mm_group(p3, wa, AB3, t0, tn, True, False)
                            ps.append((p1, p3))
                        for (p1, p3), (t0, tn) in zip(ps, defer):
                            mm_group(p1, wa, C1, t0, tn, False, True)
                            mm_group(p3, wa, C3, t0, tn, False, True)
                            consume(p1, p3, fc, t0, tn)
                        rest = t_tiles[len(defer):]
                    else:
                        wa = wpool.tile([128, 4, KD, 128], FP8, tag="wa")
                        nc.sync.dma_start(wa[:], wa_d[fc])
                        rest = t_tiles
                    for (t0, tn) in rest:
                        p1 = ps1.tile([128, tn], F32, tag="p1")
                        mm_group(p1, wa, AB1 + C1, t0, tn, True, True)
                        p3 = ps1.tile([128, tn], F32, tag="p3")
                        mm_group(p3, wa, AB3 + C3, t0, tn, True, True)
                        consume(p1, p3, fc, t0, tn)

            # Stage 2: out^T[dc] = sum_fc W2T[fc,dc]^T @ h^T[fc]
            for bi, (_, w2_d, t_tiles) in enumerate(blocks):
                for dc in range(KD):
                    w2 = w2pool.tile([128, 2, FC, 128], FP8, tag="w2")
                    nc.sync.dma_start(w2[:], w2_d[dc])
                    for (t0, tn) in t_tiles:
                        po = ps2.tile([128, tn], F32, tag="po")
                        idx = 0
                        for (w, ht_) in ((0, hh), (1, hh), (0, hl)):
                            for j in range(FC // 2):
                                nc.tensor.matmul(
                                    po[:], w2[:, w, 2 * j:2 * j + 2, :],
                                    ht_[:, 2 * j:2 * j + 2, t0:t0 + tn],
                                    start=(idx == 0), stop=(idx == 41),
                                    perf_mode=DR)
                                idx += 1
                        ot = opool.tile([128, tn], F32, tag="o")
                        nc.scalar.activation(ot[:], po[:], COPY_FN,
                                             scale=1.0 / 256)
                        nc.sync.dma_start(out_d[dc][:, t0:t0 + tn], ot[:])

    nc.compile()
    return nc


def _gate(xt, W_gate):
    """fp32 softmax top-2 gating, matching jax.lax.top_k tie-breaking."""
    logits = xt @ W_gate.T
    m = logits.max(-1, keepdims=True)
    ex = np.exp(logits - m)
    w = ex / ex.sum(-1, keepdims=True)
    top_i = np.argsort(-w, axis=-1, kind="stable")[:, :TOP_K]
    top_w = np.take_along_axis(w, top_i, -1)
    top_w = top_w / top_w.sum(-1, keepdims=True)
    return top_i, top_w.astype(np.float32)


def _split8(v):
    """hi/lo e4m3 pair: hi = fp8(v), lo = fp8(v - hi)."""
    hi = np.asarray(v, dtype=E4)
    lo = np.asarray(v - hi.astype(np.float32), dtype=E4)
    return hi, lo


def kernel(x, W_gate, W1, W3, W2):
    x = np.asarray(x, dtype=np.float32)
    W_gate = np.asarray(W_gate, dtype=np.float32)
    W1 = np.asarray(W1, dtype=np.float32)
    W3 = np.asarray(W3, dtype=np.float32)
    W2 = np.asarray(W2, dtype=np.float32)

    B, P, D = x.shape
    T = B * P
    xt = x.reshape(T, D)

    top_i, top_w = _gate(xt, W_gate)

    idxs, wts = [], []
    for e in range(NUM_EXPERTS):
        rows, slots = np.nonzero(top_i == e)
        idxs.append(rows)
        wts.append(top_w[rows, slots])

    counts = [len(i) for i in idxs]
    C, C_A, C_B, A_slots, B_slots = _plan_blocks(counts)
    if C > PASS_CAP:
        raise NotImplementedError(
            f"pathological routing (C={C}) exceeds single-pass capacity")

    wt_maps = []
    for e in range(NUM_EXPERTS):
        # lhsT tile layouts, pre-tiled on host so device DMAs are contiguous:
        # w1t[fc, dp, kd, m] = 64*W1[e][fc*128+m, kd*128+dp]
        w1t = np.ascontiguousarray(
            W1[e].T.reshape(KD, 128, FC, 128).transpose(2, 1, 0, 3)) * 64.0
        w3t = np.ascontiguousarray(
            W3[e].T.reshape(KD, 128, FC, 128).transpose(2, 1, 0, 3)) * 64.0
        # w2t[dc, fp, fc, m] = 64*W2[e][dc*128+m, fc*128+fp]
        w2t = np.ascontiguousarray(
            W2[e].T.reshape(FC, 128, KD, 128).transpose(2, 1, 0, 3)) * 64.0
        w1hi, w1lo = _split8(w1t)
        w3hi, w3lo = _split8(w3t)
        w2hi, w2lo = _split8(w2t)
        wa = np.ascontiguousarray(
            np.stack([w1hi, w1lo, w3hi, w3lo], axis=2))  # [FC,128,4,KD,128]
        w2f = np.ascontiguousarray(
            np.stack([w2hi, w2lo], axis=2))               # [KD,128,2,FC,128]
        wt_maps.append({"wa": wa, "w2": w2f})

    nc = _build_bass(C, C_A)
    out = np.zeros((T, D), dtype=np.float32)
    in_maps = []
    for core in range(NUM_EXPERTS):
        eA, sA, nA = A_slots[core]
        eB, sB, nB = B_slots[core]
        XT = np.zeros((D, C), dtype=np.float32)
        XT[:, :nA] = xt[idxs[eA][sA:sA + nA]].T
        XT[:, C_A:C_A + nB] = xt[idxs[eB][sB:sB + nB]].T
        xhi, xlo = _split8(XT)
        # device x layout: [half, dp (partition), kd', c]
        in_maps.append({
            "xhi": np.ascontiguousarray(
                xhi.reshape(2, KD // 2, 128, C).swapaxes(1, 2)),
            "xlo": np.ascontiguousarray(
                xlo.reshape(2, KD // 2, 128, C).swapaxes(1, 2)),
            "wa_a": wt_maps[eA]["wa"], "w2_a": wt_maps[eA]["w2"],
            "wa_b": wt_maps[eB]["wa"], "w2_b": wt_maps[eB]["w2"],
        })
    res = run_bass_kernel_spmd(nc, in_maps, list(range(NUM_EXPERTS)))
    LAST_RUN["results"] = res
    LAST_RUN["C"] = C
    LAST_RUN["nc"] = nc
    LAST_RUN["in_maps"] = in_maps
    for core in range(NUM_EXPERTS):
        O = np.asarray(res.results[core]["out"]).reshape(D, C)
        eA, sA, nA = A_slots[core]
        eB, sB, nB = B_slots[core]
        if nA:
            sel = idxs[eA][sA:sA + nA]
            out[sel] += wts[eA][sA:sA + nA][:, None] * O[:, :nA].T
        if nB:
            sel = idxs[eB][sB:sB + nB]
            out[sel] += wts[eB][sB:sB + nB][:, None] * O[:, C_A:C_A + nB].T
    return out.reshape(B, P, D)
